# revision 28
# baseline (speedup 1.0000x reference)
"""Trainium2 Bass kernel for nn_Attention (pooling attention).

Math (per batch b):
    u[b]     = W_score @ h_t[b]            (tiny: score = (hidden @ W_score) . h_t
                                            collapses to hidden . (W_score @ h_t))
    score[t] = hidden[b,t,:] . u[b]        (DVE fp16)
    p[t]     = exp(score[t] - 50)          (ScalarE, fused per-partition accum -> q)
    s        = sum_t p[t]                  (gpsimd partition all-reduce)
    w[t]     = p[t] / s                    (ACT copy with per-partition scale)
    ctx      = sum_t w[t] * hidden[b,t,:]  (PE fp16: w column as 1-col stationary)
    out[b]   = tanh([ctx, h_t[b]] @ W_att)

Sharding: data-parallel over batch, 16 batches per core on 8 cores; weights
replicated.  hidden_states is read from HBM exactly once (fp32), cast to fp16
during the DMA (SWDGE cast), and never transposed.

v2 changes vs baseline:
  - u broadcast via PE outer product (ones x u16) + ACT cast-copy, replacing
    2048 tiny 512B SBUF->SBUF DMA packets that halved load bandwidth.
  - W_score^T / W_att host-permuted so each partition reads one contiguous
    2KB run (128 descriptors instead of 256/512).
  - ctx scatter matmuls in fp16 (single pass) instead of fp32 LOW_HIGH.
  - w16 = p * (1/s) moved from DVE to ACT (Copy with scale AP).
  - score optionally via fused tensor_tensor_reduce (SCORE_MODE).
"""

import sys

import numpy as np

_TRN_REPO = "/opt/trn_rl_repo"
if _TRN_REPO not in sys.path:
    sys.path.insert(0, _TRN_REPO)

import concourse.bass as bass
import concourse.bacc as bacc
import concourse.tile as tile
from concourse import mybir
from concourse import bass_isa
from concourse.bass_utils import run_bass_kernel_spmd

N_CORES = 8
B, T, H = 128, 2048, 256
NB = B // N_CORES  # batches per core
P = 128  # SBUF partitions
TT = T // P  # t-tiles per batch
OUT_D = 128
EXP_SHIFT = -50.0  # keeps exp() in fp32 range; cancels in the softmax ratio

SCORE_MODE = "tree"  # "ttr" (fused tensor_tensor_reduce) or "tree"
PREF = 4  # batches of load prefetch

F32 = mybir.dt.float32
F16 = mybir.dt.float16


def _build_kernel(nc: bass.Bass, tc: "tile.TileContext", hidden, wst, watt, ident, out):
    add = mybir.AluOpType.add
    mult = mybir.AluOpType.mult

    from contextlib import ExitStack

    with ExitStack() as ctx:
        const = ctx.enter_context(tc.tile_pool(name="const", bufs=1))
        ybufs = ctx.enter_context(tc.tile_pool(name="ybufs", bufs=8))
        sc = ctx.enter_context(tc.tile_pool(name="sc", bufs=3))
        # z tree intermediates: written and consumed back-to-back on the
        # serial DVE queue, so one buffer per tag suffices
        zpool = ctx.enter_context(tc.tile_pool(name="zpool", bufs=1))
        psum_t = ctx.enter_context(tc.tile_pool(name="psum_t", bufs=2, space="PSUM"))
        psum_bc = ctx.enter_context(tc.tile_pool(name="psum_bc", bufs=2, space="PSUM"))
        psum_p = ctx.enter_context(tc.tile_pool(name="psum_p", bufs=1, space="PSUM"))

        # ---- constants / weights -------------------------------------------------
        # Prologue DMAs ride the sync (HWDGE) queue; the gpsimd (SWDGE) queue
        # carries only the 16 big y16 cast-loads, gated on wst completion so
        # the u-chain inputs land before the flood saturates the SDMAs.
        ident_sb = const.tile([16, 16], F32, tag="ident")
        nc.sync.dma_start(out=ident_sb, in_=ident[:, :])
        ht_sb = const.tile([NB, H], F32, tag="ht")
        nc.sync.dma_start(out=ht_sb, in_=hidden[:, T - 1, :])
        # wst/watt ride the gpsimd (SWDGE) queue AHEAD of the y16 flood: they
        # double as a warmup for the 16 SDMA engines so load 0 streams at
        # full rate.  host pre-permutes both so each partition reads one
        # contiguous 2KB run: wst_perm[2p+kk, h] = W_score^T[kk*128+p, h],
        # watt_perm[4p+dd, j] = W_att[dd*128+p, j]
        wst_sb = const.tile([P, 2, H], F32, tag="wst")
        nc.gpsimd.dma_start(
            out=wst_sb, in_=wst.rearrange("(p kk) h -> p kk h", kk=2)
        )
        watt_sb = const.tile([P, 4, OUT_D], F32, tag="watt")
        nc.gpsimd.dma_start(out=watt_sb, in_=watt.rearrange("(p dd) j -> p dd j", dd=4))

        ones16 = const.tile([1, P], F16, tag="ones16")
        nc.vector.memset(ones16, 1.0)
        ones_col32 = const.tile([P, 1], F32, tag="ones_col32")
        nc.vector.memset(ones_col32, 1.0)
        ones_row32 = const.tile([1, P], F32, tag="ones_row32")
        nc.vector.memset(ones_row32, 1.0)
        shift_col = const.tile([P, 1], F32, tag="shift_col")
        nc.vector.memset(shift_col, EXP_SHIFT)

        # ---- h_t, h_t^T and u = h_t @ W_score^T ---------------------------------
        htT_sb = const.tile([P, 2, NB], F32, tag="htT")  # h_t^T halves [k, half, b]
        for half in range(2):
            ps_tr = psum_t.tile([P, NB], F32, tag="ptmp", name=f"ps_tr{half}")
            nc.tensor.matmul(
                ps_tr,
                lhsT=ht_sb[:, half * P : (half + 1) * P],
                rhs=ident_sb,
                start=True,
                stop=True,
            )
            nc.scalar.copy(out=htT_sb[:, half, :], in_=ps_tr)

        ps_u = psum_t.tile([NB, H], F32, tag="ptmp")
        for half in range(2):
            nc.tensor.matmul(
                ps_u,
                lhsT=htT_sb[:, half, :],
                rhs=wst_sb[:, half, :],
                start=(half == 0),
                stop=(half == 1),
            )
        u16_sb = const.tile([NB, H], F16, tag="u16")
        nc.scalar.copy(out=u16_sb, in_=ps_u)

        # fp16 copies of h_t^T and W_att for the single-pass final matmuls
        htT16 = const.tile([P, 2, NB], F16, tag="htT16")
        nc.scalar.copy(out=htT16, in_=htT_sb)
        watt16 = const.tile([P, 4, OUT_D], F16, tag="watt16")
        nc.scalar.copy(out=watt16, in_=watt_sb)

        # per-batch broadcast of u[b] to all 128 partitions via PE: selector
        # stationary (row b ones) x u16 -> out[p, h] = u16[b, h].  ACT then
        # cast-copies PSUM->SBUF fp16.  No DMA packets, no HBM/SDMA
        # contention with the y16 load flood.
        ident16f = const.tile([NB, NB], F16, tag="ident16f")
        nc.scalar.copy(out=ident16f, in_=ident_sb)
        sel_all = const.tile([NB, NB, P], F16, tag="sel_all")
        ident_rep = bass.AP(
            tensor=ident16f.tensor,
            offset=ident16f.offset,
            ap=[list(ident16f.ap[0]), list(ident16f.ap[1]), [0, P]],
        )
        nc.vector.tensor_copy(out=sel_all, in_=ident_rep)
        ubc_all = const.tile([P, NB, H], F16, tag="ubc_all")
        for b in range(NB):
            ubc_ps = psum_bc.tile([P, H], F32, tag="ubc_ps")
            nc.tensor.matmul(
                ubc_ps,
                lhsT=sel_all[:, b, :],
                rhs=u16_sb,
                start=True,
                stop=True,
            )
            nc.scalar.copy(out=ubc_all[:, b, :], in_=ubc_ps)

        # ---- persistent PSUM accumulators ---------------------------------------
        ctxT_ps = [
            psum_p.tile([P, NB], F32, tag=f"ctxT{j}", name=f"ctxT{j}")
            for j in range(2)
        ]

        # ---- per-batch pipeline --------------------------------------------------
        # t = p*TT + i block mapping gives 16KB-contiguous runs per partition
        # (softmax/context are t-permutation-invariant, so relabeling is free).
        ylist = {}
        yld_list = {}
        TH = TT // 2  # t-tiles per half-batch (batch 0 is split for fast start)

        # batch 0 arrives as two half-loads so the DVE score chain can start
        # after only 1MB instead of 2MB
        y0 = [
            ybufs.tile([P, TH, H], F16, tag=f"y0{h}", name=f"y0_{h}")
            for h in range(2)
        ]
        yld_0a = nc.gpsimd.dma_start(
            out=y0[0], in_=hidden[0, 0 : T // 2].rearrange("(p i) h -> p i h", i=TH)
        )
        yld_0b = nc.gpsimd.dma_start(
            out=y0[1],
            in_=hidden[0, T // 2 : T].rearrange("(p i) h -> p i h", i=TH),
        )
        tile.add_dep_helper(yld_0b.ins, yld_0a.ins, reason="halves sequential")
        yld_list[0] = yld_0b
        yld_list[-1] = yld_0a

        def emit_load(k):
            y = ybufs.tile([P, TT, H], F16, tag="y16", name=f"y16_{k}")
            yld = nc.gpsimd.dma_start(
                out=y, in_=hidden[k].rearrange("(p i) h -> p i h", i=TT)
            )
            # The SWDGE ring interleaves packets of all outstanding loads;
            # unconstrained, load 0 finishes only after ~PREF transfer times.
            # Chain each load on load k-2 (counting batch-0's halves): at most
            # two interleave, engines stay saturated, the head arrives fast.
            tile.add_dep_helper(
                yld.ins, yld_list[k - 2].ins, reason="cap loads in flight at 2"
            )
            yld_list[k] = yld
            ylist[k] = y

        for k in range(1, NB):
            emit_load(k)

        # software pipelining: batch b's softmax tail (recip onward) is
        # emitted AFTER batch b+1's score chain, so the DVE FIFO never
        # stalls on the exp -> gpsimd-all-reduce round trip.
        pending = {}

        def ytile(b, i):
            # t-tile i of batch b (batch 0 lives in two half-tiles)
            if b == 0:
                return y0[i // TH][:, i % TH, :]
            return ylist[b][:, i, :]

        def score_chain(b, score, ysrc, i0, nt):
            # score[:, i0:i0+nt] = per-tile dot of y against u (mul + tree)
            ubc = ubc_all[:, b, :]
            ubc_rep = bass.AP(
                tensor=ubc.tensor,
                offset=ubc.offset,
                ap=[list(ubc.ap[0]), [0, nt], list(ubc.ap[1])],
            )
            z = zpool.tile([P, nt, H], F16, tag=f"z{nt}")
            nc.vector.tensor_mul(z, ysrc, ubc_rep)
            z1 = zpool.tile([P, nt, 128], F16, tag=f"z1_{nt}")
            nc.vector.tensor_add(z1, z[:, :, 0:128], z[:, :, 128:256])
            z2 = zpool.tile([P, nt, 64], F16, tag=f"z2_{nt}")
            nc.vector.tensor_add(z2, z1[:, :, 0:64], z1[:, :, 64:128])
            z3 = zpool.tile([P, nt, 32], F16, tag=f"z3_{nt}")
            nc.vector.tensor_add(z3, z2[:, :, 0:32], z2[:, :, 32:64])
            nc.vector.tensor_reduce(
                out=score[:, i0 : i0 + nt], in_=z3, axis=mybir.AxisListType.X, op=add
            )

        def score_phase(b):
            score = sc.tile([P, TT], F32, tag="score")
            if b == 0:
                for h in range(2):
                    score_chain(0, score, y0[h][:, :, :], h * TH, TH)
            else:
                score_chain(b, score, ylist[b][:, :, :], 0, TT)

            # p = exp(score - 50), q = per-partition sum of p
            p_t = sc.tile([P, TT], F32, tag="p")
            q = sc.tile([P, 1], F32, tag="q")
            nc.scalar.activation(
                out=p_t,
                in_=score,
                func=mybir.ActivationFunctionType.Exp,
                bias=shift_col,
                scale=1.0,
                accum_out=q,
            )

            if b >= NB - 2:
                # tail batches: gpsimd all-reduce (shorter chain; every load
                # descgen has already drained from the gpsimd FIFO by now)
                s_bc = sc.tile([P, 1], F32, tag="s_bc")
                nc.gpsimd.partition_all_reduce(s_bc, q, P, bass_isa.ReduceOp.add)
                pending[b] = ("g", p_t, s_bc)
            else:
                # s = sum over partitions of q via PE (q^T @ ones), keeping
                # the gpsimd FIFO free for pure load descgen
                s_ps = psum_p.tile([1, 1], F32, tag="s_ps")
                nc.tensor.matmul(
                    s_ps, lhsT=q, rhs=ones_col32, start=True, stop=True
                )
                s_sb = sc.tile([1, 1], F32, tag="s_sb")
                nc.scalar.copy(out=s_sb, in_=s_ps)
                pending[b] = ("p", p_t, s_sb)

        def finish_w16(b):
            # emitted after score_phase(b+1): s is long ready, so the DVE
            # recip never stalls the score stream
            mode, p_t, s_in = pending.pop(b)
            if mode == "g":
                rs_bc = sc.tile([P, 1], F32, tag="rs_bc_g")
                nc.vector.reciprocal(out=rs_bc, in_=s_in)
            else:
                rs_sb = sc.tile([1, 1], F32, tag="rs_sb")
                nc.vector.reciprocal(out=rs_sb, in_=s_in)
                # broadcast 1/s to all partitions via PE outer product
                rs_ps = psum_p.tile([P, 1], F32, tag="rs_ps")
                nc.tensor.matmul(
                    rs_ps, lhsT=ones_row32, rhs=rs_sb, start=True, stop=True
                )
                rs_bc = sc.tile([P, 1], F32, tag="rs_bc")
                nc.scalar.copy(out=rs_bc, in_=rs_ps)
            w16 = sc.tile([P, TT], F16, tag="w16")
            nc.scalar.activation(
                out=w16,
                in_=p_t,
                func=mybir.ActivationFunctionType.Copy,
                scale=rs_bc,
            )
            pending[b] = w16

        def finish_ctx(b):
            # emitted after score_phase(b+1) so its ACT copy never delays the
            # next exp / s-copy in the ACT FIFO
            w16 = pending.pop(b)
            # ctx_row = sum_t w[t]*y[t, :] (fp16 matmuls, accumulate 16 t-tiles)
            ctx_ps = psum_t.tile([1, H], F32, tag="ptmp")
            for i in range(TT):
                nc.tensor.matmul(
                    ctx_ps,
                    lhsT=w16[:, i : i + 1],
                    rhs=ytile(b, i),
                    start=(i == 0),
                    stop=(i == TT - 1),
                )
            if b > 0:
                ylist.pop(b)
            ctx_row16 = sc.tile([1, H], F16, tag="ctx_row16")
            nc.scalar.copy(out=ctx_row16, in_=ctx_ps)

            # scatter ctx_row into column b of the persistent ctx^T
            # accumulators (fp16 single-pass rank-1 matmuls)
            for j in range(2):
                nc.tensor.matmul(
                    ctxT_ps[j][:, b : b + 1],
                    lhsT=ctx_row16[:, j * P : (j + 1) * P],
                    rhs=ones16[:, 0:1],
                    start=True,
                    stop=True,
                )

        for b in range(NB):
            score_phase(b)
            if b >= 1:
                finish_w16(b - 1)
                finish_ctx(b - 1)
        finish_w16(NB - 1)
        finish_ctx(NB - 1)

        # ---- finalize: concat with h_t, @W_att, tanh ----------------------------
        # all-fp16 operands: single-pass matmuls (fp32 runs LOW/HIGH two-pass)
        preT = sc.tile([P, 2, NB], F16, tag="preT")
        for j in range(2):
            nc.scalar.copy(out=preT[:, j, :], in_=ctxT_ps[j])

        out_ps = psum_t.tile([NB, OUT_D], F32, tag="ptmp")
        for dd in range(4):
            lhsT = preT[:, dd, :] if dd < 2 else htT16[:, dd - 2, :]
            nc.tensor.matmul(
                out_ps,
                lhsT=lhsT,
                rhs=watt16[:, dd, :],
                start=(dd == 0),
                stop=(dd == 3),
            )
        out_sb = sc.tile([NB, OUT_D], F32, tag="out_sb")
        nc.scalar.activation(
            out=out_sb, in_=out_ps, func=mybir.ActivationFunctionType.Tanh
        )
        nc.sync.dma_start(out=out[:, :], in_=out_sb)


_NC_CACHE = {}


def _get_nc():
    if "nc" not in _NC_CACHE:
        nc = bacc.Bacc("TRN2", target_bir_lowering=False, debug=False)
        hidden = nc.declare_dram_parameter("hidden", [NB, T, H], F32, isOutput=False)
        wst = nc.declare_dram_parameter("w_score_t", [H, H], F32, isOutput=False)
        watt = nc.declare_dram_parameter("w_att", [2 * H, OUT_D], F32, isOutput=False)
        ident = nc.declare_dram_parameter("ident16", [16, 16], F32, isOutput=False)
        out = nc.declare_dram_parameter("out", [NB, OUT_D], F32, isOutput=True)
        with tile.TileContext(nc) as tc:
            _build_kernel(nc, tc, hidden, wst, watt, ident, out)
        nc.compile()
        _NC_CACHE["nc"] = nc
    return _NC_CACHE["nc"]


def _run(hidden_states, W_score, W_att, trace=False, trace_kwargs=None):
    hidden_states = np.ascontiguousarray(np.asarray(hidden_states, dtype=np.float32))
    W_score = np.asarray(W_score, dtype=np.float32)
    W_att = np.ascontiguousarray(np.asarray(W_att, dtype=np.float32))
    # permute rows so each SBUF partition reads one contiguous 2KB run:
    # wst_perm[2p+kk] = W_score^T[kk*128+p]; watt_perm[4p+dd] = W_att[dd*128+p]
    wst = np.ascontiguousarray(W_score.T)
    wst_perm = np.ascontiguousarray(
        wst.reshape(2, P, H).transpose(1, 0, 2).reshape(2 * P, H)
    )
    watt_perm = np.ascontiguousarray(
        W_att.reshape(4, P, OUT_D).transpose(1, 0, 2).reshape(4 * P, OUT_D)
    )
    ident = np.eye(16, dtype=np.float32)

    nc = _get_nc()
    in_maps = []
    for c in range(N_CORES):
        in_maps.append(
            {
                "hidden": hidden_states[c * NB : (c + 1) * NB],
                "w_score_t": wst_perm,
                "w_att": watt_perm,
                "ident16": ident,
            }
        )
    kwargs = {}
    if trace:
        kwargs["trace"] = True
        if trace_kwargs:
            kwargs.update(trace_kwargs)
    res = run_bass_kernel_spmd(nc, in_maps, list(range(N_CORES)), **kwargs)
    out = np.concatenate([res.results[c]["out"] for c in range(N_CORES)], axis=0)
    return out, res


def kernel(hidden_states, W_score, W_att):
    out, _ = _run(hidden_states, W_score, W_att, trace=False)
    return out
